# revision 2
# baseline (speedup 1.0000x reference)
import sys

sys.path.insert(0, "/opt/trn_rl_repo")
import numpy as np

import concourse.bacc as bacc
import concourse.mybir as mybir
import concourse.tile as tile
from concourse import bass_utils
from concourse._compat import axon_active
from concourse.masks import make_identity

f32 = mybir.dt.float32
f16 = mybir.dt.float16
bf16 = mybir.dt.bfloat16

B, H, W, C = 4, 64, 64, 512
N = H * W          # 4096 rows per batch
NOWN = N // 2      # 2048 rows owned per core
D = 64             # qk head dim
NCORES = 8

_CACHE = {}


def _build(rep=1):
    nc = bacc.Bacc(
        "TRN2", target_bir_lowering=False, debug=not axon_active(), num_devices=1
    )
    xT_d = nc.dram_tensor("xT", [C, N], f32, kind="ExternalInput").ap()
    wq_d = nc.dram_tensor("Wq", [C, D], f32, kind="ExternalInput").ap()
    wk_d = nc.dram_tensor("Wk", [C, D], f32, kind="ExternalInput").ap()
    wv_d = nc.dram_tensor("Wv", [C, C], f16, kind="ExternalInput").ap()
    xh_d = nc.dram_tensor("xh", [C, N], f16, kind="ExternalInput").ap()
    out_d = nc.dram_tensor("out", [NOWN, C], f32, kind="ExternalOutput").ap()

    X = mybir.AxisListType.X
    Exp = mybir.ActivationFunctionType.Exp
    Copy = mybir.ActivationFunctionType.Copy

    with tile.TileContext(nc) as tc:
        with tc.tile_pool(name="sb", bufs=1) as pool, tc.tile_pool(
            name="ps", bufs=1, space="PSUM"
        ) as psum:
            ident = pool.tile([128, 128], f32)
            make_identity(nc, ident)
            ones128 = pool.tile([128, 1], bf16)
            nc.vector.memset(ones128, 1.0)

            xT = pool.tile([128, 4 * N], f32)          # [cb] -> cols cb*N + j
            xT_hf = pool.tile([128, 4 * N], f16)
            v_big = pool.tile([128, 32 * C], bf16)     # [jt] -> cols jt*C + c
            qT65 = pool.tile([65, N], f32)
            kT65 = pool.tile([65, NOWN], f32)
            wqk = pool.tile([128, 4 * 128], f32)
            wv_sb = pool.tile([128, 4 * C], f16)
            negc = pool.tile([128, 16], f32)
            tmp16 = pool.tile([16, 128], f32)
            nc.vector.memset(qT65[D : D + 1, :], 1.0)

            for cb in range(4):
                nc.sync.dma_start(
                    wqk[:, cb * 128 : cb * 128 + D],
                    wq_d[cb * 128 : (cb + 1) * 128, :],
                )
                nc.sync.dma_start(
                    wqk[:, cb * 128 + D : cb * 128 + 128],
                    wk_d[cb * 128 : (cb + 1) * 128, :],
                )
                nc.sync.dma_start(
                    wv_sb[:, cb * C : (cb + 1) * C], wv_d[cb * 128 : (cb + 1) * 128, :]
                )

            with tc.For_i(0, rep, 1):
                r = 0
                # ---- load xT (pre-transposed on host) ----
                for cb in range(4):
                    nc.sync.dma_start(
                        xT[:, cb * N : (cb + 1) * N], xT_d[cb * 128 : (cb + 1) * 128, :]
                    )
                    nc.sync.dma_start(
                        xT_hf[:, cb * N : (cb + 1) * N],
                        xh_d[cb * 128 : (cb + 1) * 128, :],
                    )

                # ---- q/k projections (packed): psum rows 0..63 q, 64..127 k ----
                for g in range(3):
                    chs = list(range(g * 3, min(8, g * 3 + 3)))
                    nch = len(chs)
                    ep = psum.tile([128, 1536], f32, tag="eps", bufs=1, name=f"qk{r}_{g}")
                    for gi, ch in enumerate(chs):
                        for cb in range(4):
                            nc.tensor.matmul(
                                ep[:, gi * 512 : (gi + 1) * 512],
                                wqk[:, cb * 128 : (cb + 1) * 128],
                                xT[:, cb * N + ch * 512 : cb * N + (ch + 1) * 512],
                                start=(cb == 0),
                                stop=(cb == 3),
                            )
                    ch0 = g * 3
                    nc.vector.tensor_copy(
                        qT65[0:D, ch0 * 512 : (ch0 + nch) * 512], ep[0:D, 0 : nch * 512]
                    )
                    if ch0 < 4:
                        kn = min(4, ch0 + nch) - ch0
                        nc.vector.tensor_copy(
                            kT65[0:D, ch0 * 512 : (ch0 + kn) * 512],
                            ep[D : 2 * D, 0 : kn * 512],
                        )

                # ---- subsample max over first 512 j ----
                for t in range(16):
                    ms = psum.tile([128, 512], f32, tag="eps", bufs=1, name=f"ms{r}_{t}")
                    nc.tensor.matmul(
                        ms,
                        kT65[0:D, t * 128 : (t + 1) * 128],
                        qT65[0:D, 0:512],
                        start=True,
                        stop=True,
                    )
                    nc.vector.reduce_max(negc[:, t : t + 1], ms, axis=X, negate=True)
                ptc = psum.tile([16, 128], f32, tag="eps", bufs=1, name=f"ptc{r}")
                nc.tensor.transpose(ptc, negc, ident)
                nc.vector.tensor_copy(tmp16, ptc)
                nc.sync.dma_start(kT65[D : D + 1, :], tmp16)

                # ---- v projection (gamma folded into Wv on host) ----
                for g in range(11):
                    jbs = list(range(g * 3, min(32, g * 3 + 3)))
                    njb = len(jbs)
                    pv = psum.tile([128, 1536], f32, tag="eps", bufs=1, name=f"pv{r}_{g}")
                    for gi, jb in enumerate(jbs):
                        for cb in range(4):
                            nc.tensor.matmul(
                                pv[:, gi * 512 : (gi + 1) * 512],
                                xT_hf[:, cb * N + jb * 128 : cb * N + (jb + 1) * 128],
                                wv_sb[:, cb * C : (cb + 1) * C],
                                start=(cb == 0),
                                stop=(cb == 3),
                            )
                    jb0 = g * 3
                    nc.vector.tensor_copy(
                        v_big[:, jb0 * C : (jb0 + njb) * C], pv[:, 0 : njb * 512]
                    )

                # ---- attention over 4 i-blocks of 512 ----
                for it in range(4):
                    accv = [
                        psum.tile([128, C], f32, tag="accv", bufs=4,
                                  name=f"av{r}_{it}_{s}")
                        for s in range(4)
                    ]
                    zp = psum.tile([1, 512], f32, tag="zp", bufs=1, name=f"zp{r}_{it}")
                    for g in range(11):
                        jts = list(range(g * 3, min(32, g * 3 + 3)))
                        ep = psum.tile(
                            [128, 1536], f32, tag="eps", bufs=1, name=f"ae{r}_{it}_{g}"
                        )
                        for gi, jt in enumerate(jts):
                            nc.tensor.matmul(
                                ep[:, gi * 512 : (gi + 1) * 512],
                                qT65[:, jt * 128 : (jt + 1) * 128],
                                kT65[:, it * 512 : (it + 1) * 512],
                                start=True,
                                stop=True,
                            )
                        st = pool.tile([128, 1536], bf16, tag="st", bufs=2, name="st")
                        nc.scalar.activation(
                            st[:, 0 : len(jts) * 512], ep[:, 0 : len(jts) * 512], Exp
                        )
                        for gi, jt in enumerate(jts):
                            for s in range(4):
                                nc.tensor.matmul(
                                    accv[s],
                                    st[:, gi * 512 + s * 128 : gi * 512 + (s + 1) * 128],
                                    v_big[:, jt * C : (jt + 1) * C],
                                    start=(jt == 0),
                                    stop=(jt == 31),
                                )
                            nc.tensor.matmul(
                                zp,
                                ones128,
                                st[:, gi * 512 : (gi + 1) * 512],
                                start=(jt == 0),
                                stop=(jt == 31),
                            )

                    # z -> reciprocal column for this block, then finalize
                    zrow = pool.tile([1, 512], f32, tag="zrow", bufs=2, name="zrow")
                    nc.vector.tensor_copy(zrow, zp)
                    rtp = psum.tile([128, 4], f32, tag="zp", bufs=1, name="rtp")
                    for s in range(4):
                        nc.tensor.transpose(
                            rtp[:, s : s + 1],
                            zrow[0:1, s * 128 : (s + 1) * 128],
                            ident[0:1, 0:1],
                        )
                    rc = pool.tile([128, 4], f32, tag="rc", bufs=2, name="rc")
                    nc.vector.reciprocal(rc, rtp)
                    ob4 = pool.tile([128, 4 * C], f32, tag="ob", bufs=2, name="ob")
                    for s in range(4):
                        nc.scalar.activation(
                            ob4[:, s * C : (s + 1) * C],
                            accv[s],
                            Copy,
                            scale=rc[:, s : s + 1],
                        )
                    nc.sync.dma_start(
                        out_d[it * 512 : (it + 1) * 512, :].rearrange(
                            "(s p) c -> p s c", s=4
                        ),
                        ob4,
                    )

    nc.compile()
    return nc


def _in_maps(x, Wq, Wk, Wv, gamma):
    gamma_f = float(np.asarray(gamma).reshape(-1)[0])
    wq = np.ascontiguousarray(Wq, dtype=np.float32)
    wk = np.ascontiguousarray(Wk, dtype=np.float32)
    wvg = np.ascontiguousarray((np.asarray(Wv, dtype=np.float32) * gamma_f).astype(np.float16))
    maps = []
    for c in range(NCORES):
        b, h = c // 2, c % 2
        xb = np.asarray(x[b], dtype=np.float32).reshape(N, C)
        xr = np.roll(xb, -h * NOWN, axis=0)
        xT = np.ascontiguousarray(xr.T)
        xh = xT.astype(np.float16)
        maps.append({"xT": xT, "Wq": wq, "Wk": wk, "Wv": wvg, "xh": xh})
    return maps


def _gather(results):
    out = np.empty((B, N, C), dtype=np.float32)
    for c in range(NCORES):
        b, h = c // 2, c % 2
        out[b, h * NOWN : (h + 1) * NOWN, :] = results[c]["out"]
    return out.reshape(B, H, W, C)


def kernel(x, Wq, Wk, Wv, gamma):
    nc = _CACHE.get("nc")
    if nc is None:
        nc = _build(rep=1)
        _CACHE["nc"] = nc
    res = bass_utils.run_bass_kernel_spmd(
        nc, _in_maps(x, Wq, Wk, Wv, gamma), core_ids=list(range(NCORES))
    )
    return _gather(res.results)


# revision 3
# speedup vs baseline: 1.2864x; 1.2864x over previous
import sys

sys.path.insert(0, "/opt/trn_rl_repo")
import numpy as np

import concourse.bacc as bacc
import concourse.mybir as mybir
import concourse.tile as tile
from concourse import bass_utils
from concourse._compat import axon_active
from concourse.masks import make_identity

f32 = mybir.dt.float32
f16 = mybir.dt.float16
bf16 = mybir.dt.bfloat16

B, H, W, C = 4, 64, 64, 512
N = H * W          # 4096 rows per batch
NOWN = N // 2      # 2048 rows owned per core
D = 64             # qk head dim
NCORES = 8

_CACHE = {}


def _build(rep=1):
    nc = bacc.Bacc(
        "TRN2", target_bir_lowering=False, debug=not axon_active(), num_devices=1
    )
    xT_d = nc.dram_tensor("xT", [C, N], f32, kind="ExternalInput").ap()
    wq_d = nc.dram_tensor("Wq", [C, D], f32, kind="ExternalInput").ap()
    wk_d = nc.dram_tensor("Wk", [C, D], f32, kind="ExternalInput").ap()
    wv_d = nc.dram_tensor("Wv", [C, C], f16, kind="ExternalInput").ap()
    xh_d = nc.dram_tensor("xh", [C, N], f16, kind="ExternalInput").ap()
    out_d = nc.dram_tensor("out", [NOWN, C], f32, kind="ExternalOutput").ap()

    X = mybir.AxisListType.X
    Exp = mybir.ActivationFunctionType.Exp
    Copy = mybir.ActivationFunctionType.Copy

    with tile.TileContext(nc) as tc:
        with tc.tile_pool(name="sb", bufs=1) as pool, tc.tile_pool(
            name="ps", bufs=1, space="PSUM"
        ) as psum:
            ident = pool.tile([128, 128], f32)
            make_identity(nc, ident)
            ones128 = pool.tile([128, 1], bf16)
            nc.vector.memset(ones128, 1.0)

            xT = pool.tile([128, 4 * N], f32)          # [cb] -> cols cb*N + j
            xT_hf = pool.tile([128, 4 * N], f16)
            v_big = pool.tile([128, 32 * C], bf16)     # [jt] -> cols jt*C + c
            qT65 = pool.tile([65, N], f16)
            kT65 = pool.tile([65, NOWN], f16)
            wqk = pool.tile([128, 4 * 128], f32)
            wv_sb = pool.tile([128, 4 * C], f16)
            negc = pool.tile([128, 16], f32)
            tmp16 = pool.tile([16, 128], f16)
            nc.vector.memset(qT65[D : D + 1, :], 1.0)

            for cb in range(4):
                nc.sync.dma_start(
                    wqk[:, cb * 128 : cb * 128 + D],
                    wq_d[cb * 128 : (cb + 1) * 128, :],
                )
                nc.sync.dma_start(
                    wqk[:, cb * 128 + D : cb * 128 + 128],
                    wk_d[cb * 128 : (cb + 1) * 128, :],
                )
                nc.sync.dma_start(
                    wv_sb[:, cb * C : (cb + 1) * C], wv_d[cb * 128 : (cb + 1) * 128, :]
                )

            with tc.For_i(0, rep, 1):
                r = 0
                # ---- load xT (pre-transposed on host) ----
                for cb in range(4):
                    nc.sync.dma_start(
                        xT[:, cb * N : (cb + 1) * N], xT_d[cb * 128 : (cb + 1) * 128, :]
                    )
                    nc.sync.dma_start(
                        xT_hf[:, cb * N : (cb + 1) * N],
                        xh_d[cb * 128 : (cb + 1) * 128, :],
                    )

                # ---- q/k projections (packed): psum rows 0..63 q, 64..127 k ----
                for g in range(3):
                    chs = list(range(g * 3, min(8, g * 3 + 3)))
                    nch = len(chs)
                    ep = psum.tile([128, 1536], f32, tag="eps", bufs=1, name=f"qk{r}_{g}")
                    for gi, ch in enumerate(chs):
                        for cb in range(4):
                            nc.tensor.matmul(
                                ep[:, gi * 512 : (gi + 1) * 512],
                                wqk[:, cb * 128 : (cb + 1) * 128],
                                xT[:, cb * N + ch * 512 : cb * N + (ch + 1) * 512],
                                start=(cb == 0),
                                stop=(cb == 3),
                            )
                    ch0 = g * 3
                    nc.vector.tensor_copy(
                        qT65[0:D, ch0 * 512 : (ch0 + nch) * 512], ep[0:D, 0 : nch * 512]
                    )
                    if ch0 < 4:
                        kn = min(4, ch0 + nch) - ch0
                        nc.vector.tensor_copy(
                            kT65[0:D, ch0 * 512 : (ch0 + kn) * 512],
                            ep[D : 2 * D, 0 : kn * 512],
                        )

                # ---- subsample max over first 512 j ----
                for t in range(16):
                    ms = psum.tile([128, 512], f32, tag="eps", bufs=1, name=f"ms{r}_{t}")
                    nc.tensor.matmul(
                        ms,
                        kT65[0:D, t * 128 : (t + 1) * 128],
                        qT65[0:D, 0:512],
                        start=True,
                        stop=True,
                    )
                    nc.vector.reduce_max(negc[:, t : t + 1], ms, axis=X, negate=True)
                ptc = psum.tile([16, 128], f32, tag="eps", bufs=1, name=f"ptc{r}")
                nc.tensor.transpose(ptc, negc, ident)
                nc.vector.tensor_copy(tmp16, ptc)
                nc.sync.dma_start(kT65[D : D + 1, :], tmp16)

                # ---- v projection (gamma folded into Wv on host) ----
                for g in range(11):
                    jbs = list(range(g * 3, min(32, g * 3 + 3)))
                    njb = len(jbs)
                    pv = psum.tile([128, 1536], f32, tag="eps", bufs=1, name=f"pv{r}_{g}")
                    for gi, jb in enumerate(jbs):
                        for cb in range(4):
                            nc.tensor.matmul(
                                pv[:, gi * 512 : (gi + 1) * 512],
                                xT_hf[:, cb * N + jb * 128 : cb * N + (jb + 1) * 128],
                                wv_sb[:, cb * C : (cb + 1) * C],
                                start=(cb == 0),
                                stop=(cb == 3),
                            )
                    jb0 = g * 3
                    nc.vector.tensor_copy(
                        v_big[:, jb0 * C : (jb0 + njb) * C], pv[:, 0 : njb * 512]
                    )

                # ---- attention over 4 i-blocks of 512 ----
                for it in range(4):
                    accv = [
                        psum.tile([128, C], f32, tag="accv", bufs=4,
                                  name=f"av{r}_{it}_{s}")
                        for s in range(4)
                    ]
                    zp = psum.tile([1, 512], f32, tag="zp", bufs=1, name=f"zp{r}_{it}")
                    for g in range(11):
                        jts = list(range(g * 3, min(32, g * 3 + 3)))
                        ep = psum.tile(
                            [128, 1536], f32, tag="eps", bufs=1, name=f"ae{r}_{it}_{g}"
                        )
                        for gi, jt in enumerate(jts):
                            nc.tensor.matmul(
                                ep[:, gi * 512 : (gi + 1) * 512],
                                qT65[:, jt * 128 : (jt + 1) * 128],
                                kT65[:, it * 512 : (it + 1) * 512],
                                start=True,
                                stop=True,
                            )
                        st = pool.tile([128, 1536], bf16, tag="st", bufs=2, name="st")
                        nc.scalar.activation(
                            st[:, 0 : len(jts) * 512], ep[:, 0 : len(jts) * 512], Exp
                        )
                        for gi, jt in enumerate(jts):
                            for s in range(4):
                                nc.tensor.matmul(
                                    accv[s],
                                    st[:, gi * 512 + s * 128 : gi * 512 + (s + 1) * 128],
                                    v_big[:, jt * C : (jt + 1) * C],
                                    start=(jt == 0),
                                    stop=(jt == 31),
                                )
                            nc.tensor.matmul(
                                zp,
                                ones128,
                                st[:, gi * 512 : (gi + 1) * 512],
                                start=(jt == 0),
                                stop=(jt == 31),
                            )

                    # z -> reciprocal column for this block, then finalize
                    zrow = pool.tile([1, 512], f32, tag="zrow", bufs=2, name="zrow")
                    nc.vector.tensor_copy(zrow, zp)
                    rtp = psum.tile([128, 4], f32, tag="zp", bufs=1, name="rtp")
                    for s in range(4):
                        nc.tensor.transpose(
                            rtp[:, s : s + 1],
                            zrow[0:1, s * 128 : (s + 1) * 128],
                            ident[0:1, 0:1],
                        )
                    rc = pool.tile([128, 4], f32, tag="rc", bufs=2, name="rc")
                    nc.vector.reciprocal(rc, rtp)
                    ob4 = pool.tile([128, 4 * C], f32, tag="ob", bufs=2, name="ob")
                    for s in range(4):
                        nc.scalar.activation(
                            ob4[:, s * C : (s + 1) * C],
                            accv[s],
                            Copy,
                            scale=rc[:, s : s + 1],
                        )
                    nc.sync.dma_start(
                        out_d[it * 512 : (it + 1) * 512, :].rearrange(
                            "(s p) c -> p s c", s=4
                        ),
                        ob4,
                    )

    nc.compile()
    return nc


def _in_maps(x, Wq, Wk, Wv, gamma):
    gamma_f = float(np.asarray(gamma).reshape(-1)[0])
    wq = np.ascontiguousarray(Wq, dtype=np.float32)
    wk = np.ascontiguousarray(Wk, dtype=np.float32)
    wvg = np.ascontiguousarray((np.asarray(Wv, dtype=np.float32) * gamma_f).astype(np.float16))
    maps = []
    for c in range(NCORES):
        b, h = c // 2, c % 2
        xb = np.asarray(x[b], dtype=np.float32).reshape(N, C)
        xr = np.roll(xb, -h * NOWN, axis=0)
        xT = np.ascontiguousarray(xr.T)
        xh = xT.astype(np.float16)
        maps.append({"xT": xT, "Wq": wq, "Wk": wk, "Wv": wvg, "xh": xh})
    return maps


def _gather(results):
    out = np.empty((B, N, C), dtype=np.float32)
    for c in range(NCORES):
        b, h = c // 2, c % 2
        out[b, h * NOWN : (h + 1) * NOWN, :] = results[c]["out"]
    return out.reshape(B, H, W, C)


def kernel(x, Wq, Wk, Wv, gamma):
    nc = _CACHE.get("nc")
    if nc is None:
        nc = _build(rep=1)
        _CACHE["nc"] = nc
    res = bass_utils.run_bass_kernel_spmd(
        nc, _in_maps(x, Wq, Wk, Wv, gamma), core_ids=list(range(NCORES))
    )
    return _gather(res.results)
